# revision 5
# baseline (speedup 1.0000x reference)
"""Trainium2 Bass kernel for nn_ChannelMixing (RWKV-style channel mixing).

Math: the reference's FFT decay-conv is the first-order IIR
    h[t] = mix*h[t-1] + x[t],  h[-1] = last_x/(1-mix)
and x_mix = (1-mix)*h, so with weights pre-scaled by (1-mix):
    k = h_k @ (Wk*(1-mix_k)).T,  r = h_r @ (Wr*(1-mix_r)).T
    out = sigmoid(r) * (relu(k)^2 @ Wv.T)

Sharding: time dimension L=4096 split over 8 cores (512 rows each) with a
128-step halo to warm up the scan state (decay <= sigmoid(1) ~ 0.731, so
carry across 128 steps < 1e-17 — below fp32 noise). Core 0 gets the exact
initial state via a per-core init column; no collectives.

Layout: everything [channel(P), time(F)]. The scan runs on the vector
engine (tensor_tensor_scan), the three 2048x2048 matmuls on the PE in
fp32r, activations on ACT, gating on DVE.
"""
import numpy as np
from contextlib import ExitStack

import concourse.bass as bass
from concourse import bacc
import concourse.tile as tile
import concourse.mybir as mybir
from concourse.bass_utils import run_bass_kernel_spmd

LEN, DIM = 4096, 2048
NCORES = 8
P = 128
HALO = 128

f32 = mybir.dt.float32
f32r = mybir.dt.float32r
Alu = mybir.AluOpType
Act = mybir.ActivationFunctionType

_cache = {}


def _build(dim, tloc, halo):
    """Build + compile the per-core SPMD program."""
    nt = dim // P          # channel tiles
    ts = tloc + halo       # time slab incl. halo
    ng = max(1, (dim // P) // 4)   # output m-groups of 4 m-tiles
    NF = 512 if tloc >= 512 else tloc   # matmul moving size (time)
    assert tloc % NF == 0
    nf = tloc // NF        # time blocks per matmul (1 at full size)

    nc = bacc.Bacc(trn_type="TRN2", debug=False)

    xs_d = nc.dram_tensor("xs", [dim, ts], f32, kind="ExternalInput").ap()
    dec_d = nc.dram_tensor("dec", [dim, 2], f32, kind="ExternalInput").ap()   # col0=mix_k col1=mix_r
    ini_d = nc.dram_tensor("ini", [dim, 2], f32, kind="ExternalInput").ap()   # col0=h0_k col1=h0_r
    wk_d = nc.dram_tensor("wk", [dim, dim], f32r, kind="ExternalInput").ap()  # [d, i] pre-scaled
    wr_d = nc.dram_tensor("wr", [dim, dim], f32r, kind="ExternalInput").ap()
    wv_d = nc.dram_tensor("wv", [dim, dim], f32r, kind="ExternalInput").ap()  # [i, o]
    out_d = nc.dram_tensor("out", [dim, tloc], f32, kind="ExternalOutput").ap()

    with tile.TileContext(nc) as tc, ExitStack() as ctx:
        const = ctx.enter_context(tc.tile_pool(name="const", bufs=1))
        xs_pool = ctx.enter_context(tc.tile_pool(name="xs", bufs=4))
        h_pool = ctx.enter_context(tc.tile_pool(name="h", bufs=1))
        w_pool = ctx.enter_context(tc.tile_pool(name="w", bufs=4))
        ev_pool = ctx.enter_context(tc.tile_pool(name="ev", bufs=1))
        sc_pool = ctx.enter_context(tc.tile_pool(name="sc", bufs=4))
        o_pool = ctx.enter_context(tc.tile_pool(name="o", bufs=4))
        ps_pool = ctx.enter_context(tc.tile_pool(name="ps", bufs=2, space="PSUM"))

        # per-channel constants: [P, nt] tiles (col ct = chan tile ct)
        dec_t = const.tile([P, 2 * nt], f32)
        nc.sync.dma_start(dec_t[:].rearrange("p (ct c) -> p ct c", c=2),
                          dec_d.rearrange("(ct p) c -> p ct c", p=P))
        ini_t = const.tile([P, 2 * nt], f32)
        nc.sync.dma_start(ini_t[:].rearrange("p (ct c) -> p ct c", c=2),
                          ini_d.rearrange("(ct p) c -> p ct c", p=P))

        # ---- stage A: decay scans -> h_k, h_r in [chan, time] ----
        h = {"k": [], "r": []}
        for p in ("k", "r"):
            for ct in range(nt):
                h[p].append(h_pool.tile([P, tloc], f32r, tag=f"h{p}{ct}", name=f"h{p}{ct}"))
        for ct in range(nt):
            xs = xs_pool.tile([P, ts], f32, tag="xs")
            nc.sync.dma_start(xs[:], xs_d[ct * P:(ct + 1) * P, :])
            for pi, p in enumerate(("k", "r")):
                dcol = dec_t[:, 2 * ct + pi: 2 * ct + pi + 1]
                # halo scan from 0 state
                hh = sc_pool.tile([P, halo], f32, tag="hh")
                nc.vector.tensor_tensor_scan(
                    hh[:], dcol.broadcast_to([P, halo]), xs[:, :halo], 0.0,
                    op0=Alu.mult, op1=Alu.add)
                # combined init: halo tail + per-core override (core0 halo is 0)
                ic = sc_pool.tile([P, 1], f32, tag="ic")
                nc.vector.tensor_add(ic[:], hh[:, halo - 1: halo],
                                     ini_t[:, 2 * ct + pi: 2 * ct + pi + 1])
                nc.vector.tensor_tensor_scan(
                    h[p][ct][:], dcol.broadcast_to([P, tloc]), xs[:, halo:], ic[:],
                    op0=Alu.mult, op1=Alu.add)

        # ---- stage B helper: out[o_tile, t] = sum_kt w[kt,o].T @ rhs[kt] ----
        def big_matmul(w_dram, rhs_tiles, evict_fn, wtag):
            for g in range(ng):
                m4 = min(4, nt - 4 * g)
                psums = [ps_pool.tile([P, NF], f32, tag=f"ps{m}", name=f"ps_{wtag}_{g}_{m}") for m in range(m4)]
                for tb in range(nf):
                    for kt in range(nt):
                        wt = w_pool.tile([P, m4 * P], f32r, tag=wtag)
                        nc.sync.dma_start(
                            wt[:], w_dram[kt * P:(kt + 1) * P,
                                          g * 4 * P: g * 4 * P + m4 * P])
                        for m in range(m4):
                            nc.tensor.matmul(
                                psums[m][:], wt[:, m * P:(m + 1) * P],
                                rhs_tiles[kt][:, tb * NF:(tb + 1) * NF],
                                start=(kt == 0), stop=(kt == nt - 1))
                    for m in range(m4):
                        evict_fn(g * 4 + m, tb, psums[m])

        # k path: evict = relu then square -> sq tiles (f32r)
        sq = [ev_pool.tile([P, tloc], f32r, tag=f"sq{i}", name=f"sq{i}") for i in range(nt)]

        def evict_k(mi, tb, psum):
            rr = sc_pool.tile([P, NF], f32, tag="rr")
            nc.scalar.activation(rr[:], psum[:], Act.Relu)
            nc.vector.tensor_mul(sq[mi][:, tb * NF:(tb + 1) * NF], rr[:], rr[:])

        # r path: evict = sigmoid -> sig tiles (f32)
        sig = [ev_pool.tile([P, tloc], f32, tag=f"sg{i}", name=f"sg{i}") for i in range(nt)]

        def evict_r(mi, tb, psum):
            nc.scalar.activation(sig[mi][:, tb * NF:(tb + 1) * NF], psum[:], Act.Sigmoid)

        # v path: evict = gate with sigmoid(r) -> DMA out
        def evict_v(mi, tb, psum):
            ot = o_pool.tile([P, NF], f32, tag="ot")
            nc.vector.tensor_mul(ot[:], psum[:], sig[mi][:, tb * NF:(tb + 1) * NF])
            nc.sync.dma_start(out_d[mi * P:(mi + 1) * P, tb * NF:(tb + 1) * NF], ot[:])

        big_matmul(wk_d, h["k"], evict_k, "wk")
        big_matmul(wr_d, h["r"], evict_r, "wr")
        big_matmul(wv_d, sq, evict_v, "wv")

    nc.compile()
    return nc


def _sigmoid(v):
    return 1.0 / (1.0 + np.exp(-v.astype(np.float64)))


def _prep(x, Wk, Wr, Wv, mix_k, mix_r, lxk, lxr, ncores, halo):
    """Host-side prep: transposes, weight pre-scaling, per-core slabs."""
    dim = x.shape[1]
    tloc = x.shape[0] // ncores
    mk = _sigmoid(mix_k).astype(np.float32)
    mr = _sigmoid(mix_r).astype(np.float32)
    dec = np.stack([mk, mr], axis=1)                       # [dim, 2]
    h0k = (lxk / (1.0 - mk)).astype(np.float32)
    h0r = (lxr / (1.0 - mr)).astype(np.float32)

    wk = np.ascontiguousarray((Wk * (1.0 - mk)[None, :]).T.astype(np.float32))
    wr = np.ascontiguousarray((Wr * (1.0 - mr)[None, :]).T.astype(np.float32))
    wv = np.ascontiguousarray(Wv.T.astype(np.float32))

    xT = np.ascontiguousarray(x.T.astype(np.float32))       # [dim, L]
    in_maps = []
    for c in range(ncores):
        t0 = c * tloc
        slab = np.empty((dim, halo + tloc), np.float32)
        if c == 0:
            slab[:, :halo] = 0.0
            ini = np.stack([h0k, h0r], axis=1)
        else:
            slab[:, :halo] = xT[:, t0 - halo: t0]
            ini = np.zeros((dim, 2), np.float32)
        slab[:, halo:] = xT[:, t0: t0 + tloc]
        in_maps.append({
            "xs": slab, "dec": dec, "ini": np.ascontiguousarray(ini),
            "wk": wk, "wr": wr, "wv": wv,
        })
    return in_maps


def kernel(x, Wk, Wr, Wv, mix_k, mix_r, last_x_mix_k, last_x_mix_r):
    x = np.asarray(x, np.float32)
    Wk = np.asarray(Wk, np.float32)
    Wr = np.asarray(Wr, np.float32)
    Wv = np.asarray(Wv, np.float32)
    mix_k = np.asarray(mix_k, np.float32)
    mix_r = np.asarray(mix_r, np.float32)
    lxk = np.asarray(last_x_mix_k, np.float32)
    lxr = np.asarray(last_x_mix_r, np.float32)

    L, dim = x.shape
    tloc = L // NCORES
    key = (dim, tloc, HALO)
    if key not in _cache:
        _cache[key] = _build(dim, tloc, HALO)
    nc = _cache[key]

    in_maps = _prep(x, Wk, Wr, Wv, mix_k, mix_r, lxk, lxr, NCORES, HALO)
    # First execution on a cold device occasionally returns
    # NRT_EXEC_UNIT_UNRECOVERABLE; a retry has always succeeded.
    res = None
    for attempt in range(3):
        try:
            res = run_bass_kernel_spmd(nc, in_maps, core_ids=list(range(NCORES)))
            break
        except Exception:
            if attempt == 2:
                raise

    out = np.empty((L, dim), np.float32)
    for c in range(NCORES):
        out[c * tloc:(c + 1) * tloc, :] = res.results[c]["out"].T
    return out


# revision 7
# speedup vs baseline: 1.3191x; 1.3191x over previous
"""Trainium2 Bass kernel for nn_ChannelMixing (RWKV-style channel mixing).

Math: the reference's FFT decay-conv is the first-order IIR
    h[t] = mix*h[t-1] + x[t],  h[-1] = last_x/(1-mix)
and x_mix = (1-mix)*h, so with weights pre-scaled by (1-mix):
    k = h_k @ (Wk*(1-mix_k)).T,  r = h_r @ (Wr*(1-mix_r)).T
    out = sigmoid(r) * (relu(k)^2 @ Wv.T)

Sharding: time dimension L=4096 split over 8 cores (512 rows each) with a
64-step halo to warm up the scan state (decay <= sigmoid(1) ~ 0.731, so
carry across 64 steps < 3e-9 — below fp32 noise). Core 0 gets the exact
initial state via a per-core init column; no collectives.

Layout: everything [channel(P), time(F)]. The scan runs on the vector
engine (tensor_tensor_scan), the three 2048x2048 matmuls on the PE in
fp32r, activations on ACT, gating on DVE.
"""
import numpy as np
from contextlib import ExitStack

import concourse.bass as bass
from concourse import bacc
import concourse.tile as tile
import concourse.mybir as mybir
from concourse.bass_utils import run_bass_kernel_spmd

LEN, DIM = 4096, 2048
NCORES = 8
P = 128
HALO = 64

f32 = mybir.dt.float32
f32r = mybir.dt.float32r
Alu = mybir.AluOpType
Act = mybir.ActivationFunctionType

_cache = {}


def _build(dim, tloc, halo):
    """Build + compile the per-core SPMD program."""
    nt = dim // P          # channel tiles
    ts = tloc + halo       # time slab incl. halo
    ng = max(1, (dim // P) // 4)   # output m-groups of 4 m-tiles
    NF = 512 if tloc >= 512 else tloc   # matmul moving size (time)
    assert tloc % NF == 0
    nf = tloc // NF        # time blocks per matmul (1 at full size)

    nc = bacc.Bacc(trn_type="TRN2", debug=False)

    xs_d = nc.dram_tensor("xs", [dim, ts], f32, kind="ExternalInput").ap()
    dec_d = nc.dram_tensor("dec", [dim, 2], f32, kind="ExternalInput").ap()   # col0=mix_k col1=mix_r
    ini_d = nc.dram_tensor("ini", [dim, 2], f32, kind="ExternalInput").ap()   # col0=h0_k col1=h0_r
    wk_d = nc.dram_tensor("wk", [dim, dim], f32r, kind="ExternalInput").ap()  # [d, i] pre-scaled
    wr_d = nc.dram_tensor("wr", [dim, dim], f32r, kind="ExternalInput").ap()
    wv_d = nc.dram_tensor("wv", [dim, dim], f32r, kind="ExternalInput").ap()  # [i, o]
    out_d = nc.dram_tensor("out", [dim, tloc], f32, kind="ExternalOutput").ap()

    with tile.TileContext(nc) as tc, ExitStack() as ctx:
        const = ctx.enter_context(tc.tile_pool(name="const", bufs=1))
        xs_pool = ctx.enter_context(tc.tile_pool(name="xs", bufs=6))
        h_pool = ctx.enter_context(tc.tile_pool(name="h", bufs=1))
        w_pool = ctx.enter_context(tc.tile_pool(name="w", bufs=12))
        ev_pool = ctx.enter_context(tc.tile_pool(name="ev", bufs=1))
        sc_pool = ctx.enter_context(tc.tile_pool(name="sc", bufs=3))
        o_pool = ctx.enter_context(tc.tile_pool(name="o", bufs=3))
        ps_pool = ctx.enter_context(tc.tile_pool(name="ps", bufs=2, space="PSUM"))

        # per-channel constants: [P, nt] tiles (col ct = chan tile ct)
        dec_t = const.tile([P, 2 * nt], f32)
        nc.sync.dma_start(dec_t[:].rearrange("p (ct c) -> p ct c", c=2),
                          dec_d.rearrange("(ct p) c -> p ct c", p=P))
        ini_t = const.tile([P, 2 * nt], f32)
        nc.sync.dma_start(ini_t[:].rearrange("p (ct c) -> p ct c", c=2),
                          ini_d.rearrange("(ct p) c -> p ct c", p=P))

        # ---- stage A: decay scans -> h_k, h_r in [chan, time] ----
        h = {"k": [], "r": []}
        for p in ("k", "r"):
            for ct in range(nt):
                h[p].append(h_pool.tile([P, tloc], f32r, tag=f"h{p}{ct}", name=f"h{p}{ct}"))
        for pi, p in enumerate(("k", "r")):
            for ct in range(nt):
                xs = xs_pool.tile([P, ts], f32, tag="xs", name=f"xs{p}{ct}")
                nc.sync.dma_start(xs[:], xs_d[ct * P:(ct + 1) * P, :])
                dcol = dec_t[:, 2 * ct + pi: 2 * ct + pi + 1]
                # halo scan from 0 state
                hh = sc_pool.tile([P, halo], f32, tag="hh")
                nc.vector.tensor_tensor_scan(
                    hh[:], dcol.broadcast_to([P, halo]), xs[:, :halo], 0.0,
                    op0=Alu.mult, op1=Alu.add)
                # combined init: halo tail + per-core override (core0 halo is 0)
                ic = sc_pool.tile([P, 1], f32, tag="ic")
                nc.vector.tensor_add(ic[:], hh[:, halo - 1: halo],
                                     ini_t[:, 2 * ct + pi: 2 * ct + pi + 1])
                nc.vector.tensor_tensor_scan(
                    h[p][ct][:], dcol.broadcast_to([P, tloc]), xs[:, halo:], ic[:],
                    op0=Alu.mult, op1=Alu.add)

        # ---- stage B helper: out[o_tile, t] = sum_kt w[kt,o].T @ rhs[kt] ----
        def big_matmul(w_dram, rhs_tiles, evict_fn, wtag):
            for g in range(ng):
                m4 = min(4, nt - 4 * g)
                psums = [ps_pool.tile([P, NF], f32, tag=f"ps{m}", name=f"ps_{wtag}_{g}_{m}") for m in range(m4)]
                for tb in range(nf):
                    for kt in range(nt):
                        wt = w_pool.tile([P, m4 * P], f32r, tag="w", name=f"wt_{wtag}_{g}_{kt}")
                        nc.sync.dma_start(
                            wt[:], w_dram[kt * P:(kt + 1) * P,
                                          g * 4 * P: g * 4 * P + m4 * P])
                        for m in range(m4):
                            nc.tensor.matmul(
                                psums[m][:], wt[:, m * P:(m + 1) * P],
                                rhs_tiles[kt][:, tb * NF:(tb + 1) * NF],
                                start=(kt == 0), stop=(kt == nt - 1))
                    for m in range(m4):
                        evict_fn(g * 4 + m, tb, psums[m])

        # k path: evict = relu then square -> sq tiles (f32r)
        sq = [ev_pool.tile([P, tloc], f32r, tag=f"sq{i}", name=f"sq{i}") for i in range(nt)]

        def evict_k(mi, tb, psum):
            rr = sc_pool.tile([P, NF], f32, tag="rr")
            nc.scalar.activation(rr[:], psum[:], Act.Relu)
            nc.vector.tensor_mul(sq[mi][:, tb * NF:(tb + 1) * NF], rr[:], rr[:])

        # r path: evict = sigmoid -> sig tiles (f32)
        sig = [ev_pool.tile([P, tloc], f32, tag=f"sg{i}", name=f"sg{i}") for i in range(nt)]

        def evict_r(mi, tb, psum):
            nc.scalar.activation(sig[mi][:, tb * NF:(tb + 1) * NF], psum[:], Act.Sigmoid)

        # v path: evict = gate with sigmoid(r) -> DMA out
        def evict_v(mi, tb, psum):
            ot = o_pool.tile([P, NF], f32, tag="ot")
            nc.vector.tensor_mul(ot[:], psum[:], sig[mi][:, tb * NF:(tb + 1) * NF])
            nc.sync.dma_start(out_d[mi * P:(mi + 1) * P, tb * NF:(tb + 1) * NF], ot[:])

        big_matmul(wk_d, h["k"], evict_k, "wk")
        big_matmul(wr_d, h["r"], evict_r, "wr")
        big_matmul(wv_d, sq, evict_v, "wv")

    nc.compile()
    return nc


def _sigmoid(v):
    return 1.0 / (1.0 + np.exp(-v.astype(np.float64)))


def _prep(x, Wk, Wr, Wv, mix_k, mix_r, lxk, lxr, ncores, halo):
    """Host-side prep: transposes, weight pre-scaling, per-core slabs."""
    dim = x.shape[1]
    tloc = x.shape[0] // ncores
    mk = _sigmoid(mix_k).astype(np.float32)
    mr = _sigmoid(mix_r).astype(np.float32)
    dec = np.stack([mk, mr], axis=1)                       # [dim, 2]
    h0k = (lxk / (1.0 - mk)).astype(np.float32)
    h0r = (lxr / (1.0 - mr)).astype(np.float32)

    wk = np.ascontiguousarray((Wk * (1.0 - mk)[None, :]).T.astype(np.float32))
    wr = np.ascontiguousarray((Wr * (1.0 - mr)[None, :]).T.astype(np.float32))
    wv = np.ascontiguousarray(Wv.T.astype(np.float32))

    xT = np.ascontiguousarray(x.T.astype(np.float32))       # [dim, L]
    in_maps = []
    for c in range(ncores):
        t0 = c * tloc
        slab = np.empty((dim, halo + tloc), np.float32)
        if c == 0:
            slab[:, :halo] = 0.0
            ini = np.stack([h0k, h0r], axis=1)
        else:
            slab[:, :halo] = xT[:, t0 - halo: t0]
            ini = np.zeros((dim, 2), np.float32)
        slab[:, halo:] = xT[:, t0: t0 + tloc]
        in_maps.append({
            "xs": slab, "dec": dec, "ini": np.ascontiguousarray(ini),
            "wk": wk, "wr": wr, "wv": wv,
        })
    return in_maps


def kernel(x, Wk, Wr, Wv, mix_k, mix_r, last_x_mix_k, last_x_mix_r):
    x = np.asarray(x, np.float32)
    Wk = np.asarray(Wk, np.float32)
    Wr = np.asarray(Wr, np.float32)
    Wv = np.asarray(Wv, np.float32)
    mix_k = np.asarray(mix_k, np.float32)
    mix_r = np.asarray(mix_r, np.float32)
    lxk = np.asarray(last_x_mix_k, np.float32)
    lxr = np.asarray(last_x_mix_r, np.float32)

    L, dim = x.shape
    tloc = L // NCORES
    key = (dim, tloc, HALO)
    if key not in _cache:
        _cache[key] = _build(dim, tloc, HALO)
    nc = _cache[key]

    in_maps = _prep(x, Wk, Wr, Wv, mix_k, mix_r, lxk, lxr, NCORES, HALO)
    # First execution on a cold device occasionally returns
    # NRT_EXEC_UNIT_UNRECOVERABLE; a retry has always succeeded.
    res = None
    for attempt in range(3):
        try:
            res = run_bass_kernel_spmd(nc, in_maps, core_ids=list(range(NCORES)))
            break
        except Exception:
            if attempt == 2:
                raise

    out = np.empty((L, dim), np.float32)
    for c in range(NCORES):
        out[c * tloc:(c + 1) * tloc, :] = res.results[c]["out"].T
    return out
